# revision 1
# baseline (speedup 1.0000x reference)
"""Trainium2 Bass kernel for nn_BandpassFilter (first-order Butterworth
band-pass: high-pass(low_cutoff) + low-pass(high_cutoff), summed).

Math
----
The reference runs two coupled first-order IIR filters over T=262144 time
steps per waveform:  y[n] = b0*x[n] + b1*x[n-1] - a1*y[n-1]  (per filter,
zero initial state), output = y_hp + y_lp.

The combined impulse response is h[0] = bh0 + bl0 and, for d >= 1,
h[d] = ch*Ah^(d-1) + cl*Al^(d-1)  with  Af = -af1, cf = bf1 - af1*bf0.
|Ah| ~ 0.972, |Al| ~ 0.867 for the given cutoffs, so h decays below the
fp32 noise floor by d ~ 500.  The IIR therefore equals (to fp32 accuracy)
a causal FIR convolution with ~512 taps — which maps onto the TensorEngine
with NO sequential scan via a polyphase decomposition:

  t = 128*j + p   (p = phase = partition, j = column)
  y[q, j] = sum_{m=0..3} sum_p  Wm[q, p] * x[p, j - m]
  Wm[q, p] = h[q - p + 128*m]          (taps d in [0, 511])

i.e. 4 accumulating [128k x 128m x 512n] matmuls per 512-column tile,
contraction over the 128 phases.  Shifted columns (j-m) are plain AP
offsets into the phase-major SBUF buffer (zero-padded on the left, which
reproduces the zero initial conditions exactly).

Layout: phase-major requires a 128x128 transpose on the way in and out;
both are done on the TensorEngine (transpose mode), with PSUM->SBUF
copies split between VectorE and ScalarE.  All matmuls/transposes run as
float32r (full-rate fp32 mode of the PE).

Sharding: batch dim (64 waveforms) split 8 ways across the 8 NeuronCores;
the filter is per-waveform so there is no cross-core communication.
"""

import numpy as np

SAMPLE_RATE = 44100.0
B_FULL = 64
T = 262144
NCORES = 8
RPC = B_FULL // NCORES  # rows (waveforms) per core
P = 128                 # phases == partitions
J = T // P              # 2048 phase-major columns per row
JC = J // P             # 16 column-chunks of 128 (transpose granularity)
NTILE = J // 512        # 4 conv tiles of 512 columns
NLAGS = 4               # m = 0..3  ->  taps d in [0, 511]
PAD = 4                 # left zero-padding columns (>= NLAGS-1; 4 for ISA-friendly memset width)


def _coeffs(low_cutoff, high_cutoff):
    """butter(1, wn) coefficients, mirroring the fp32 arithmetic of the
    reference (bilinear transform)."""
    f32 = np.float32
    nyq = f32(SAMPLE_RATE / 2.0)
    low = np.clip(f32(low_cutoff), f32(0.0), nyq)
    high = np.clip(f32(high_cutoff), low, nyq)

    def butter1(wn, btype):
        t = np.tan(f32(np.pi) * wn / f32(2.0))
        a1 = (t - f32(1.0)) / (t + f32(1.0))
        if btype == "low":
            b0 = t / (f32(1.0) + t)
            b1 = b0
        else:
            b0 = f32(1.0) / (f32(1.0) + t)
            b1 = -b0
        return b0, b1, a1

    bh0, bh1, ah1 = butter1(low / nyq, "high")
    bl0, bl1, al1 = butter1(high / nyq, "low")
    return (bh0, bh1, ah1), (bl0, bl1, al1)


def _impulse_response(low_cutoff, high_cutoff, n):
    (bh0, bh1, ah1), (bl0, bl1, al1) = _coeffs(low_cutoff, high_cutoff)
    # exact powers in float64 of the fp32 coefficients
    Ah, Al = -np.float64(ah1), -np.float64(al1)
    ch = np.float64(bh1) - np.float64(ah1) * np.float64(bh0)
    cl = np.float64(bl1) - np.float64(al1) * np.float64(bl0)
    d = np.arange(1, n)
    h = np.empty(n, np.float64)
    h[0] = np.float64(bh0) + np.float64(bl0)
    h[1:] = ch * Ah ** (d - 1) + cl * Al ** (d - 1)
    return h


def _weights(low_cutoff, high_cutoff):
    """W tensor, already transposed for the matmul's lhsT operand:
    w[m, p, q] = h[q - p + 128*m]  (zero where the index is negative)."""
    h = _impulse_response(low_cutoff, high_cutoff, NLAGS * P)
    q = np.arange(P)[None, :]
    p = np.arange(P)[:, None]
    w = np.zeros((NLAGS, P, P), np.float64)
    for m in range(NLAGS):
        d = q - p + P * m
        valid = d >= 0
        w[m][valid] = h[d[valid]]
    return w.astype(np.float32)


_BUILD_CACHE = {}


def _legalize_waits(nc, mybir):
    """This walrus build accepts at most ONE sync-wait per instruction.
    Tile emits several on some instructions (DMA lane FIFO + slot release
    etc.); split the extras into standalone single-wait EventSemaphore
    instructions on the same engine queue, which preserves ordering."""
    n = 0
    for fn in nc.m.functions:
        for blk in fn.blocks:
            new = []
            for inst in blk.instructions:
                si = getattr(inst, "sync_info", None)
                if si is not None and si.on_wait and len(si.on_wait) > 1:
                    waits = list(si.on_wait)
                    for w in waits[:-1]:
                        n += 1
                        new.append(mybir.InstEventSemaphore(
                            name=f"wsplit-{n}-{inst.name}",
                            engine=inst.engine,
                            ins=[], outs=[],
                            sync_info=mybir.SyncInfo(on_wait=[w],
                                                     on_update=[]),
                        ))
                    inst.sync_info = mybir.SyncInfo(
                        on_wait=[waits[-1]],
                        on_update=list(si.on_update or []))
                new.append(inst)
            blk.instructions = new
    return n


def build_nc(reps=1, legalize=True, loop_n=1):
    """Build the per-core Bass program (identical on all 8 cores).
    loop_n > 1 wraps the body in a hardware For_i loop (timing builds)."""
    key = (reps, legalize, loop_n)
    if key in _BUILD_CACHE:
        return _BUILD_CACHE[key]

    import concourse.bass as bass
    import concourse.mybir as mybir
    from concourse import tile
    from contextlib import ExitStack

    f32 = mybir.dt.float32
    f32r = mybir.dt.float32r

    nc = bass.Bass()
    x_in = nc.declare_dram_parameter("x", [RPC, T], f32, isOutput=False)
    w_in = nc.declare_dram_parameter("w", [NLAGS, P, P], f32, isOutput=False)
    id_in = nc.declare_dram_parameter("ident", [P, P], f32, isOutput=False)
    zp_in = nc.declare_dram_parameter("zpad", [P, PAD], f32, isOutput=False)
    y_out = nc.declare_dram_parameter("y", [RPC, T], f32, isOutput=True)

    with tile.TileContext(nc) as tc, ExitStack() as ctx:
        const = ctx.enter_context(tc.tile_pool(name="const", bufs=1))
        xn_pool = ctx.enter_context(tc.tile_pool(name="xn", bufs=2))
        xt_pool = ctx.enter_context(tc.tile_pool(name="xt", bufs=2))
        xl_pool = ctx.enter_context(tc.tile_pool(name="xl", bufs=2))
        ys_pool = ctx.enter_context(tc.tile_pool(name="ys", bufs=2))
        yt_pool = ctx.enter_context(tc.tile_pool(name="yt", bufs=2))
        psi_pool = ctx.enter_context(
            tc.tile_pool(name="psi", bufs=2, space="PSUM"))
        psy_pool = ctx.enter_context(
            tc.tile_pool(name="psy", bufs=4, space="PSUM"))
        pso_pool = ctx.enter_context(
            tc.tile_pool(name="pso", bufs=2, space="PSUM"))

        # Full-precision weights come in as fp32; the fp32r hi/lo split is
        # done ON DEVICE so the split matches the hardware's own f32r
        # rounding exactly:  w_h = f32r(w),  w0_l = f32r(w0 - w0_h).
        w_f = const.tile([P, NLAGS * P], f32)   # [p, (m q)] full fp32
        w_h = const.tile([P, NLAGS * P], f32r)  # f32r-rounded weights
        w0l = const.tile([P, P], f32r)          # m=0 residual
        w0d = const.tile([P, P], f32)           # fp32 scratch for residual
        id_f = const.tile([P, P], f32)          # identity for transposes
        nc.scalar.dma_start(out=id_f[:], in_=id_in[:])
        nc.scalar.dma_start(
            out=w_f[:].rearrange("p (m q) -> p m q", q=P),
            in_=w_in.rearrange("m p q -> p m q"),
        )
        zc = const.tile([P, PAD], f32r)
        nc.gpsimd.dma_start(out=zc[:], in_=zp_in[:])
        nc.vector.tensor_copy(w_h[:], w_f[:])              # fp32 -> f32r round
        nc.vector.tensor_sub(w0d[:], w_f[:, 0:P], w_h[:, 0:P])
        nc.vector.tensor_copy(w0l[:], w0d[:])              # residual -> f32r

        # warm-up: absorb each constant-DMA semaphore tick into the PE
        # vector clock with single-wait instructions.  Every fp32r matmul
        # self-loads its weights, so the lowered instruction has exactly
        # ONE sync-wait slot; bf16 dummy LDWEIGHTS ops ("pe_dep") absorb
        # cross-engine ticks so real matmuls only carry their PSUM-bank
        # WAW wait.
        bf16 = mybir.dt.bfloat16

        def pe_dep(ap):
            nc.tensor.ldweights(ap.bitcast(bf16))

        warm_f = pso_pool.tile([P, 512], f32, tag="pso")
        nc.tensor.transpose(warm_f[:, 0:P], id_f[:], id_f[:])
        warm_y = psy_pool.tile([P, 512], f32, tag="psy")
        nc.tensor.matmul(warm_y[:, 0:P], w_h[:, 0:P], w_h[:, 0:P],
                         start=True, stop=True)
        pe_dep(w0l[:, 0:1])

        def copy_dve(out, in_):
            return nc.vector.tensor_copy(out, in_)

        def copy_act(out, in_):
            return nc.scalar.copy(out, in_)

        prev_yt = None
        if loop_n > 1:
            ctx.enter_context(tc.For_i(0, loop_n, 1, staggered_reset=True))
        for rep in range(reps):
            for r in range(RPC):
                # ---- load (natural layout: partition = j%128, free = (c,p))
                xn = xn_pool.tile([P, J], f32, tag="xn")
                xn3 = xn[:].rearrange("j (c p) -> j c p", p=P)
                xr3 = x_in[r].rearrange("(c j p) -> j c p", j=P, p=P)
                if r == 0:
                    # first row: quarter DMAs so the very first transposes
                    # start a quarter-transfer earlier
                    qc = JC // 4
                    for g in range(4):
                        nc.sync.dma_start(out=xn3[:, g * qc:(g + 1) * qc],
                                          in_=xr3[:, g * qc:(g + 1) * qc])
                else:
                    nc.sync.dma_start(out=xn3[:, 0:JC // 2],
                                      in_=xr3[:, 0:JC // 2])
                    nc.sync.dma_start(out=xn3[:, JC // 2:JC],
                                      in_=xr3[:, JC // 2:JC])
                xt = xt_pool.tile([P, PAD + J], f32r, tag="xt")
                xl = xl_pool.tile([P, PAD + J], f32r, tag="xl")
                nc.vector.tensor_copy(xt[:, 0:PAD], zc[:])
                nc.vector.tensor_copy(xl[:, 0:PAD], zc[:])
                ys = ys_pool.tile([P, J], f32, tag="ys")
                yt = yt_pool.tile([P, J], f32, tag="yt")

                pe_dep(xn[:, 0:1])  # absorb the x DMA tick

                def transpose_group(g):
                    psi = psi_pool.tile([P, 512], f32, tag="psi")
                    for k in range(4):
                        jc = g * 4 + k
                        nc.tensor.transpose(
                            psi[:, k * P:(k + 1) * P],
                            xn[:, jc * P:(jc + 1) * P],
                            id_f[:],
                        )
                    sl = slice(PAD + g * 512, PAD + (g + 1) * 512)
                    copy_dve(xt[:, sl], psi[:])          # fp32 -> f32r (hi)
                    nc.vector.tensor_sub(xl[:, sl], psi[:], xt[:, sl])

                def conv_tile(jt):
                    pe_dep(xl[:, PAD + jt * 512:PAD + jt * 512 + 1])
                    psy = psy_pool.tile([P, 512], f32, tag="psy")
                    b0 = PAD + jt * 512
                    nc.tensor.matmul(psy[:], w_h[:, 0:P],
                                     xt[:, b0:b0 + 512],
                                     start=True, stop=False)
                    nc.tensor.matmul(psy[:], w_h[:, 0:P],
                                     xl[:, b0:b0 + 512],
                                     start=False, stop=False)
                    nc.tensor.matmul(psy[:], w0l[:],
                                     xt[:, b0:b0 + 512],
                                     start=False, stop=False)
                    for m in range(1, NLAGS):
                        base = b0 - m
                        nc.tensor.matmul(
                            psy[:],
                            w_h[:, m * P:(m + 1) * P],
                            xt[:, base:base + 512],
                            start=False,
                            stop=(m == NLAGS - 1),
                        )
                    copy_dve(ys[:, jt * 512:(jt + 1) * 512], psy[:])

                def out_group(g):
                    pe_dep(ys[:, g * 512:g * 512 + 1])
                    if g >= 2:
                        # pso slot reuse within the row: absorb the
                        # out-copy (DVE) release tick too
                        pe_dep(yt[:, (g - 2) * 512:(g - 2) * 512 + 1])
                    elif prev_yt is not None:
                        # pso slot reuse across rows: absorb the previous
                        # row's out-copy g+2 release tick
                        pe_dep(prev_yt[:, (g + 2) * 512:(g + 2) * 512 + 1])
                    pso = pso_pool.tile([P, 512], f32, tag="pso")
                    for k in range(4):
                        jb = g * 4 + k
                        nc.tensor.transpose(
                            pso[:, k * P:(k + 1) * P],
                            ys[:, jb * P:(jb + 1) * P],
                            id_f[:],
                        )
                    copy_act(yt[:, g * 512:(g + 1) * 512], pso[:])

                transpose_group(0)
                transpose_group(1)
                conv_tile(0)
                transpose_group(2)
                conv_tile(1)
                transpose_group(3)
                conv_tile(2)
                conv_tile(3)
                yo3 = y_out[r].rearrange("(c j q) -> j c q", j=P, q=P)
                yt3 = yt[:].rearrange("j (c q) -> j c q", q=P)
                if r < RPC - 1:
                    out_group(0)
                    out_group(1)
                    nc.scalar.dma_start(out=yo3[:, 0:JC // 2],
                                        in_=yt3[:, 0:JC // 2])
                    out_group(2)
                    out_group(3)
                    nc.scalar.dma_start(out=yo3[:, JC // 2:JC],
                                        in_=yt3[:, JC // 2:JC])
                else:
                    # last row: quarter DMAs on the (idle) SP ring so the
                    # kernel tail ends ~a quarter-transfer after the final
                    # out-copy instead of a half-transfer behind ACT
                    qc = JC // 4
                    for g in range(4):
                        out_group(g)
                        nc.sync.dma_start(
                            out=yo3[:, g * qc:(g + 1) * qc],
                            in_=yt3[:, g * qc:(g + 1) * qc])
                prev_yt = yt

    if legalize:
        _legalize_waits(nc, mybir)
    _BUILD_CACHE[key] = nc
    return nc


def kernel(x, low_cutoff, high_cutoff):
    from concourse.bass_utils import run_bass_kernel_spmd

    x = np.asarray(x, dtype=np.float32)
    w = _weights(np.asarray(low_cutoff), np.asarray(high_cutoff))
    ident = np.eye(P, dtype=np.float32)

    nc = build_nc(reps=1)
    in_maps = [
        {"x": np.ascontiguousarray(x[c * RPC:(c + 1) * RPC]),
         "w": w, "ident": ident, "zpad": np.zeros((P, PAD), np.float32)}
        for c in range(NCORES)
    ]
    res = run_bass_kernel_spmd(nc, in_maps, list(range(NCORES)))
    return np.concatenate([res.results[c]["y"] for c in range(NCORES)], axis=0)



# revision 24
# speedup vs baseline: 2.7155x; 2.7155x over previous
"""Trainium2 Bass kernel for nn_BandpassFilter (first-order Butterworth
band-pass: high-pass(low_cutoff) + low-pass(high_cutoff), summed).

Math
----
The reference runs two coupled first-order IIR filters over T=262144 time
steps per waveform:  y[n] = b0*x[n] + b1*x[n-1] - a1*y[n-1]  (per filter,
zero initial state), output = y_hp + y_lp.

The combined impulse response is h[0] = bh0 + bl0 and, for d >= 1,
h[d] = ch*Ah^(d-1) + cl*Al^(d-1)  with  Af = -af1, cf = bf1 - af1*bf0.
|Ah| ~ 0.972, |Al| ~ 0.867 for the given cutoffs; truncating h at 256
taps leaves a residual ~1e-3 of the output scale — far inside the 2e-2
gate.  The IIR therefore becomes a causal 256-tap FIR, mapped onto the
TensorEngine with NO sequential scan via a polyphase decomposition.

With t = 128*M + p (p = phase, M = column) and M = 128*c + j:

  y[j, (c q)] = sum_{m=0..1} sum_p  xt[p, 128c+j-m] * Wm[q, p]
  Wm[q, p] = h[q - p + 128*m]          (taps d in [0, 255])

Each 128-column chunk c is ONE pair of accumulating fp16 matmuls with
lhsT = the xt chunk (so the OUTPUT comes out directly in natural layout,
partition = j) and rhs = the 128x128 tap matrix.  This folds the output
transposition into the convolution itself: there is no separate
out-transpose pass and no PSUM->SBUF conversion of a phase-major result.
The lag term (m=1) is the same matmul with the lhsT window slid one
column left (zero-padded at the row start = exact zero initial state).

Engine/queue plan (cost-model driven)
-------------------------------------
The simulator charges each DMA to its ISSUING engine queue (free bytes
per partition x 0.3855 ns) with queues running concurrently, so the
kernel splits both the input and output streams across the Pool and SP
queues and software-pipelines with decoupled lookahead (input DMAs 3
rows ahead, transposes 1 row ahead) so every queue's program order is
monotone in pipeline stage:

  Pool (gpsimd): casting DMA HBM fp32 -> SBUF fp16 for input half 0
                 (only gpsimd may cast in flight) + output half 1
  SP   :         fp32 input half 1 + output half 0
  PE   :         fp16 in-transposes (1 cyc/row) + fp16 conv matmuls
  DVE  :         fp32->fp16 convert (2x SBUF mode), PSUM->SBUF fp16
                 copies (2x mode), pad memset
  ACT  :         PSUM->SBUF fp32 output copies

Measured end-to-end error ~2e-3 max-rel vs the 2e-2 gate.

Sharding: batch dim (64 waveforms) split 8 ways across the 8 NeuronCores;
the filter is per-waveform so there is no cross-core communication.
"""

import numpy as np

SAMPLE_RATE = 44100.0
B_FULL = 64
T = 262144
NCORES = 8
RPC = B_FULL // NCORES  # rows (waveforms) per core
P = 128                 # phases == partitions
J = T // P              # 2048 phase-major columns per row
JC = J // P             # 16 column-chunks of 128
NLAGS = 2               # m = 0..1  ->  taps d in [0, 255]
PAD = 4                 # left zero-padding columns (>= NLAGS-1)
H = J // 2              # 1024: half-row columns
DMA_AHEAD = 2           # input DMA lookahead (rows)
XF_AHEAD = 1            # transpose lookahead (rows)
PSU_BUFS = 6            # PSUM conv-output buffering ([P,512] tiles)
USE_PE_DEPS = False     # ldweights tick absorbers (scheduler hoists them badly)
WARM_TILES = 3          # dummy PE warm-up tile groups (p-state ramp)
YT_SPLIT = False        # yt copies: False = both ACT, True = DVE/ACT
YT_BUFS = 2             # yt SBUF buffers
OUT_SPLIT = 2           # output stream queues (2: SP/Pool halves)


def _coeffs(low_cutoff, high_cutoff):
    """butter(1, wn) coefficients, mirroring the fp32 arithmetic of the
    reference (bilinear transform)."""
    f32 = np.float32
    nyq = f32(SAMPLE_RATE / 2.0)
    low = np.clip(f32(low_cutoff), f32(0.0), nyq)
    high = np.clip(f32(high_cutoff), low, nyq)

    def butter1(wn, btype):
        t = np.tan(f32(np.pi) * wn / f32(2.0))
        a1 = (t - f32(1.0)) / (t + f32(1.0))
        if btype == "low":
            b0 = t / (f32(1.0) + t)
            b1 = b0
        else:
            b0 = f32(1.0) / (f32(1.0) + t)
            b1 = -b0
        return b0, b1, a1

    bh0, bh1, ah1 = butter1(low / nyq, "high")
    bl0, bl1, al1 = butter1(high / nyq, "low")
    return (bh0, bh1, ah1), (bl0, bl1, al1)


def _impulse_response(low_cutoff, high_cutoff, n):
    (bh0, bh1, ah1), (bl0, bl1, al1) = _coeffs(low_cutoff, high_cutoff)
    # exact powers in float64 of the fp32 coefficients
    Ah, Al = -np.float64(ah1), -np.float64(al1)
    ch = np.float64(bh1) - np.float64(ah1) * np.float64(bh0)
    cl = np.float64(bl1) - np.float64(al1) * np.float64(bl0)
    d = np.arange(1, n)
    h = np.empty(n, np.float64)
    h[0] = np.float64(bh0) + np.float64(bl0)
    h[1:] = ch * Ah ** (d - 1) + cl * Al ** (d - 1)
    return h


def _weights(low_cutoff, high_cutoff):
    """Tap matrices used as the matmul's rhs (moving) operand, laid out
    [p, (m q)]:  w[p, m*P + q] = h[q - p + 128*m]  (zero where the tap
    index is negative).  fp16."""
    h = _impulse_response(low_cutoff, high_cutoff, NLAGS * P)
    q = np.arange(P)[None, :]
    p = np.arange(P)[:, None]
    w = np.zeros((P, NLAGS * P), np.float64)
    for m in range(NLAGS):
        d = q - p + P * m
        valid = d >= 0
        w[:, m * P:(m + 1) * P][valid] = h[d[valid]]
    return w.astype(np.float16)


_BUILD_CACHE = {}


def _legalize_waits(nc, mybir):
    """This walrus build accepts at most ONE sync-wait per instruction.
    Tile emits several on some instructions (DMA lane FIFO + slot release
    etc.); split the extras into standalone single-wait EventSemaphore
    instructions on the same engine queue, which preserves ordering."""
    n = 0
    for fn in nc.m.functions:
        for blk in fn.blocks:
            new = []
            for inst in blk.instructions:
                si = getattr(inst, "sync_info", None)
                if si is not None and si.on_wait and len(si.on_wait) > 1:
                    waits = list(si.on_wait)
                    for w in waits[:-1]:
                        n += 1
                        new.append(mybir.InstEventSemaphore(
                            name=f"wsplit-{n}-{inst.name}",
                            engine=inst.engine,
                            ins=[], outs=[],
                            sync_info=mybir.SyncInfo(on_wait=[w],
                                                     on_update=[]),
                        ))
                    inst.sync_info = mybir.SyncInfo(
                        on_wait=[waits[-1]],
                        on_update=list(si.on_update or []))
                new.append(inst)
            blk.instructions = new
    return n


def build_nc(reps=1, legalize=True, loop_n=1):
    """Build the per-core Bass program (identical on all 8 cores).
    loop_n > 1 wraps the body in a hardware For_i loop (timing builds)."""
    key = (reps, legalize, loop_n)
    if key in _BUILD_CACHE:
        return _BUILD_CACHE[key]

    import concourse.bass as bass
    import concourse.mybir as mybir
    from concourse import tile
    from contextlib import ExitStack

    f32 = mybir.dt.float32
    f16 = mybir.dt.float16
    bf16 = mybir.dt.bfloat16

    nc = bass.Bass()
    x_in = nc.declare_dram_parameter("x", [RPC, T], f32, isOutput=False)
    w_in = nc.declare_dram_parameter("w", [P, NLAGS * P], f16, isOutput=False)
    idh_in = nc.declare_dram_parameter("identh", [P, P], f16, isOutput=False)
    y_out = nc.declare_dram_parameter("y", [RPC, T], f32, isOutput=True)

    with tile.TileContext(nc) as tc, ExitStack() as ctx:
        const = ctx.enter_context(tc.tile_pool(name="const", bufs=1))
        xnh_pool = ctx.enter_context(tc.tile_pool(name="xnh", bufs=DMA_AHEAD))
        xnf_pool = ctx.enter_context(tc.tile_pool(name="xnf", bufs=DMA_AHEAD))
        xh2_pool = ctx.enter_context(tc.tile_pool(name="xh2", bufs=2))
        xt_pool = ctx.enter_context(tc.tile_pool(name="xt", bufs=3))
        psi_pool = ctx.enter_context(
            tc.tile_pool(name="psi", bufs=2, space="PSUM"))
        psu_pool = ctx.enter_context(
            tc.tile_pool(name="psu", bufs=PSU_BUFS, space="PSUM"))

        id_h = const.tile([P, P], f16)         # identity for f16 transposes
        w_t = const.tile([P, NLAGS * P], f16)  # fp16 rhs taps [p, (m q)]
        warm_c = const.tile([P, P], f32)       # ACT warm-up copy target
        nc.scalar.dma_start(out=id_h[:], in_=idh_in[:])
        nc.scalar.dma_start(out=w_t[:], in_=w_in[:])

        # warm-up: absorb each constant-DMA semaphore tick into the PE
        # vector clock with single-wait instructions, eat the one-time ACT
        # activation-table load (~1.3us), and keep the PE busy through its
        # p-state ramp (full clock needs ~3us of continuous execution)
        # while the first input DMAs are still in flight.
        def pe_dep(ap):
            if USE_PE_DEPS:
                nc.tensor.ldweights(ap.bitcast(bf16))

        dummy = const.tile([P, P], f16)   # zeroed; warm outputs unread
        nc.vector.memset(dummy[:], 0.0)
        nc.scalar.copy(warm_c[:], id_h[:])
        for wi in range(2 * WARM_TILES):
            warm_y = psu_pool.tile([P, 512], f32, tag="psu")
            for k in range(4):
                nc.tensor.matmul(warm_y[:, k * P:(k + 1) * P],
                                 dummy[:], dummy[:],
                                 start=True, stop=True)

        def copy_dve(out, in_):
            return nc.vector.tensor_copy(out, in_)

        def copy_act(out, in_):
            return nc.scalar.copy(out, in_)

        from concourse.bass import balance_dma_aps, MAX_DMA_LAST_DIM, \
            shorten_engine_name

        def psum_dma(eng, out, in_):
            """dma_start for a PSUM source (bass's helper only allows
            SBUF/DRAM; the DGE and the simulator handle PSUM reads fine)."""
            out2, in2 = balance_dma_aps(
                out, in_, max_dma_last_dim=MAX_DMA_LAST_DIM,
                allow_non_contiguous_reason=nc._allow_non_contiguous_dma_reason)
            out_ap = eng.lower_ap_dma(out2)
            in_ap = eng.lower_ap_dma(in2)
            if eng.engine in nc.hwdge_engines:
                qn = f"q{shorten_engine_name(eng.engine.name)}DynamicHW"
            else:
                qn = f"q{eng.engine.name}Dynamic"
            return eng.add_instruction(mybir.InstDMACopy(
                name=nc.get_next_instruction_name(),
                queue=qn, mode="Copy",
                ins=[*in_ap], outs=[*out_ap],
                oob_is_err=True, cce_op=mybir.AluOpType.bypass,
                bass_cond_hint=None, single_packet=False,
            ))

        if loop_n > 1:
            ctx.enter_context(tc.For_i(0, loop_n, 1, staggered_reset=True))

        state = {}

        def load_dma(r):
            """Input DMAs for row r: casting half 0 on Pool, fp32 half 1
            on SP.  Natural layout: partition = j%128, free = (c,p)."""
            xnh = xnh_pool.tile([P, H], f16, tag="xnh")
            xnf = xnf_pool.tile([P, H], f32, tag="xnf")
            xr3 = x_in[r].rearrange("(c j p) -> j c p", j=P, p=P)
            hc = JC // 2
            xh3 = xnh[:].rearrange("j (c p) -> j c p", p=P)
            xf3 = xnf[:].rearrange("j (c p) -> j c p", p=P)
            if r == 0:
                # first row: quarters so the very first transposes start
                # a quarter-transfer earlier
                nc.gpsimd.dma_start(out=xh3[:, 0:4], in_=xr3[:, 0:4])
                nc.gpsimd.dma_start(out=xh3[:, 4:8], in_=xr3[:, 4:8])
            else:
                nc.gpsimd.dma_start(out=xh3[:], in_=xr3[:, 0:hc])
            nc.sync.dma_start(out=xf3[:, 0:4], in_=xr3[:, hc:hc + 4])
            nc.scalar.dma_start(out=xf3[:, 4:8], in_=xr3[:, hc + 4:JC])
            state[("in", r)] = (xnh, xnf)

        def load_xform(r):
            """Phase-major transposition for row r (PE fp16 transposes +
            DVE copies); half 1 is first converted fp32->fp16 on DVE."""
            xnh, xnf = state.pop(("in", r))
            xh2 = xh2_pool.tile([P, H], f16, tag="xh2")
            copy_dve(xh2[:], xnf[:])  # fp32 -> fp16, SBUF 2x mode
            xt = xt_pool.tile([P, PAD + J], f16, tag="xt")
            nc.vector.memset(xt[:, 0:PAD], 0.0)
            prev_xt = state.get("xt")
            for h in range(2):
                src = xnh if h == 0 else xh2
                pe_dep(src[:, 0:1])
                if prev_xt is not None:
                    # psi slot reuse: absorb the previous row's xt-copy
                    # release tick (same parity -> same DVE position)
                    pe_dep(prev_xt[:, PAD + h * H:PAD + h * H + 1])
                psi = psi_pool.tile([P, H], f16, tag="psi")
                for k in range(8):
                    nc.tensor.transpose(
                        psi[:, k * P:(k + 1) * P],
                        src[:, k * P:(k + 1) * P],
                        id_h[:],
                    )
                copy_dve(xt[:, PAD + h * H:PAD + (h + 1) * H], psi[:])
            state["xt"] = xt
            state[("xt", r)] = xt

        def comp(r):
            """Convolution for row r: per 128-column chunk, 2 accumulating
            fp16 matmuls with lhsT = xt chunk (output lands in natural
            layout in PSUM), then the output is DMAd STRAIGHT from PSUM to
            HBM in 2-chunk pieces spread over the SP/Pool/ACT queues."""
            xt = state.pop(("xt", r))
            yo3 = y_out[r].rearrange("(c j q) -> j c q", j=P, q=P)
            for t in range(4):
                psu = psu_pool.tile([P, 512], f32, tag="psu")
                psu3 = psu[:].rearrange("j (c q) -> j c q", q=P)
                for k in range(4):
                    c = 4 * t + k
                    b0 = PAD + c * P
                    sl = psu[:, k * P:(k + 1) * P]
                    nc.tensor.matmul(sl, xt[:, b0:b0 + P],
                                     w_t[:, 0:P],
                                     start=True, stop=False)
                    nc.tensor.matmul(sl, xt[:, b0 - 1:b0 + P - 1],
                                     w_t[:, P:2 * P],
                                     start=False, stop=True)
                c0 = 4 * t
                q2 = nc.gpsimd if t % 2 == 0 else nc.scalar
                psum_dma(nc.sync, yo3[:, c0:c0 + 2], psu3[:, 0:2])
                psum_dma(q2, yo3[:, c0 + 2:c0 + 4], psu3[:, 2:4])

        for rep in range(reps):
            # decoupled software pipeline: input DMAs run DMA_AHEAD rows
            # ahead (greedy, before outputs on the same queues), the
            # transposes XF_AHEAD rows ahead, so no queue ever convoys a
            # future row's early stage behind the current row's tail.
            for r in range(min(DMA_AHEAD, RPC)):
                load_dma(r)
            for r in range(min(XF_AHEAD, RPC)):
                load_xform(r)
            for r in range(RPC):
                if r + DMA_AHEAD < RPC:
                    load_dma(r + DMA_AHEAD)
                if r + XF_AHEAD < RPC:
                    load_xform(r + XF_AHEAD)
                comp(r)

    if legalize:
        _legalize_waits(nc, mybir)
    _BUILD_CACHE[key] = nc
    return nc


def kernel(x, low_cutoff, high_cutoff):
    from concourse.bass_utils import run_bass_kernel_spmd

    x = np.asarray(x, dtype=np.float32)
    w = _weights(np.asarray(low_cutoff), np.asarray(high_cutoff))
    identh = np.eye(P, dtype=np.float16)

    nc = build_nc(reps=1)
    in_maps = [
        {"x": np.ascontiguousarray(x[c * RPC:(c + 1) * RPC]),
         "w": w, "identh": identh}
        for c in range(NCORES)
    ]
    res = run_bass_kernel_spmd(nc, in_maps, list(range(NCORES)))
    return np.concatenate([res.results[c]["y"] for c in range(NCORES)], axis=0)
